# revision 14
# baseline (speedup 1.0000x reference)
"""Cross-attention Trainium2 kernel (8 NeuronCores, SPMD, no collectives).

Reference computation (f32):
    Q = tokens @ Wq; K = context @ Wk; V = context @ Wv    (per batch)
    attn = softmax(Q K^T / sqrt(64)); ctx = attn V; out = ctx @ Wo + bo

Sharding: the flattened (B*T = 16384) token rows are split into 8 slices of
2048; each slice lies inside a single batch, so each core computes its own
batch's K/V locally (context is small) and needs no cross-core traffic.

Layouts: all activations contraction-major; scores transposed ([S, T]); the
softmax row-sum comes from a ones-column appended to V.  All matmul inputs
bf16 (fp8 was measured at 2-3e-2 max-rel error per stage -- over budget).

v4 schedule: the PE is the bottleneck (~375us of issue slots: scores 88,
PV 111, projections 152, K/V 42).  The Scalar engine (exp, 286us) runs
underneath.  All projection matmuls are fillers packed into the score/PV
stream via a ns-credit-paced queue whose due-dates double as emission-order
(=Tile dependency) correctness bounds; scores are emitted one chunk ahead
so ACT keeps >=1 chunk of backlog across group boundaries; inputs stream in
tn-chunk order on three parallel DMA rings so the first score matmul can
issue ~6us after launch.
"""

import numpy as np
import ml_dtypes

import concourse.bass as bass
import concourse.mybir as mybir
import concourse.tile as tile
from concourse import bacc
from concourse.bass_utils import run_bass_kernel_spmd

# problem shapes (hardcoded per the contract)
B, T, S = 4, 4096, 1024
HID, EMB, CTX, H = 1024, 1024, 768, 16
D = EMB // H  # 64
N_CORES = 8
TC = (B * T) // N_CORES  # 2048 token rows per core

F32 = mybir.dt.float32
BF16 = mybir.dt.bfloat16
AF = mybir.ActivationFunctionType

_BUILT = {}


def _build_nc(tc_=TC, s_=S, ctx_=CTX, hid_=HID, h_=H, num_cores=N_CORES):
    nc = bacc.Bacc("TRN2", target_bir_lowering=False, debug=False,
                   num_devices=num_cores)
    emb_ = hid_

    tokT = nc.dram_tensor("tokT", [hid_, tc_], BF16, kind="ExternalInput")
    ctxT = nc.dram_tensor("ctxT", [ctx_, s_], BF16, kind="ExternalInput")
    wq = nc.dram_tensor("wq", [hid_, emb_], BF16, kind="ExternalInput")
    wk = nc.dram_tensor("wk", [ctx_, emb_], BF16, kind="ExternalInput")
    wv = nc.dram_tensor("wv", [ctx_, hid_], BF16, kind="ExternalInput")
    wo = nc.dram_tensor("wo", [emb_, hid_], BF16, kind="ExternalInput")
    bo = nc.dram_tensor("bo", [1, hid_], BF16, kind="ExternalInput")
    out = nc.dram_tensor("out", [tc_, hid_], F32, kind="ExternalOutput")

    K8 = hid_ // 128   # contraction chunks for Q proj
    C6 = ctx_ // 128   # contraction chunks for K/V proj
    E8 = emb_ // 128   # embedding chunks
    S8 = s_ // 128     # source-sequence chunks
    TN = tc_ // 512    # T chunks per core
    NH = hid_ // 512   # output free chunks
    HID, EMB, CTX, S, TC, H = hid_, emb_, ctx_, s_, tc_, h_  # noqa: shadow
    NP = H // 2        # head pairs per T chunk
    NG = TN * NP       # total groups

    with tile.TileContext(nc) as tc:
        with (
            tc.tile_pool(name="const", bufs=1) as const,
            tc.tile_pool(name="qpool", bufs=2) as qpool,
            tc.tile_pool(name="cpool", bufs=2) as cpool,
            tc.tile_pool(name="attn", bufs=9) as attnp,
            tc.tile_pool(name="small", bufs=2) as small,
            tc.tile_pool(name="ostage", bufs=2) as ostage,
            tc.tile_pool(name="spsum", bufs=2, space="PSUM") as spsum,
            tc.tile_pool(name="cpsum", bufs=2, space="PSUM") as cpsum,
            tc.tile_pool(name="ppsum", bufs=2, space="PSUM") as ppsum,
        ):
            # ---- resident inputs (single big tiles, 3 DMA rings) ------
            # Prologue-critical data (ctx+wk for kproj, wq+tok-tn0 for
            # qproj, 6MB total) is spread over all three rings so it lands
            # ~13us in.  The scalar ring carries ONLY wq: its 8 trigger
            # instructions retire ~5us in, long before the first exp
            # ACTIVATE (~19us), so the Scalar engine is never blocked.
            wq16 = const.tile([128, K8, EMB], BF16)
            for k in range(K8):
                nc.scalar.dma_start(out=wq16[:, k, :],
                                    in_=wq[k * 128:(k + 1) * 128, :])
            ctx16 = const.tile([128, C6, S], BF16)
            for c in range(C6):
                nc.sync.dma_start(out=ctx16[:, c, :],
                                  in_=ctxT[c * 128:(c + 1) * 128, :])
            wk16 = const.tile([128, C6, EMB], BF16)
            for c in range(C6):
                nc.gpsimd.dma_start(out=wk16[:, c, :],
                                    in_=wk[c * 128:(c + 1) * 128, :])
            tok16 = const.tile([128, K8, TC], BF16)
            for k in range(K8):
                eng = nc.sync if k < K8 // 2 else nc.gpsimd
                eng.dma_start(
                    out=tok16[:, k, 0:512],
                    in_=tokT[k * 128:(k + 1) * 128, 0:512])
            for tn in range(1, TN):
                for k in range(K8):
                    nc.sync.dma_start(
                        out=tok16[:, k, tn * 512:(tn + 1) * 512],
                        in_=tokT[k * 128:(k + 1) * 128,
                                 tn * 512:(tn + 1) * 512])
            wv16 = const.tile([128, C6, HID], BF16)
            for c in range(C6):
                nc.gpsimd.dma_start(out=wv16[:, c, :],
                                    in_=wv[c * 128:(c + 1) * 128, :])
            wo16 = const.tile([128, E8, HID], BF16)
            for e in range(E8):
                nc.gpsimd.dma_start(out=wo16[:, e, :],
                                    in_=wo[e * 128:(e + 1) * 128, :])
            bo_bc = const.tile([128, HID], BF16)
            nc.gpsimd.dma_start(out=bo_bc, in_=bo[0:1, :].to_broadcast((128, HID)))

            # ---- SBUF result tiles ------------------------------------
            kT_sb = const.tile([128, E8, S], BF16)
            # V layout: [128, s8, pair, 192] = [V_par0 | ones | V_par1];
            # par-p lhsT is the overlapping window cols par*64 .. par*64+128,
            # so both parities share one ones block
            v_sb = const.tile([128, S8, NP, 3 * D], BF16)
            for s in range(S8):
                nc.vector.memset(v_sb[:, s, :, D:2 * D], 1.0)

            qts = [qpool.tile([128, E8, 512], BF16, tag="qt", name=f"qt{i}")
                   for i in range(2)]
            cts = [cpool.tile([128, E8, 512], BF16, tag="ct", name=f"ct{i}")
                   for i in range(2)]

            # ---- work units (cost in ~ns of PE issue time) ------------
            def kproj_unit(e):
                """kT_sb[:, e, :] = (context @ Wk)^T e-chunk."""
                for n in range(S // 512):
                    ps = ppsum.tile([128, 512], F32, tag="proj")
                    for c in range(C6):
                        nc.tensor.matmul(
                            ps,
                            lhsT=wk16[:, c, e * 128:(e + 1) * 128],
                            rhs=ctx16[:, c, n * 512:(n + 1) * 512],
                            start=(c == 0), stop=(c == C6 - 1),
                        )
                    nc.vector.tensor_copy(kT_sb[:, e, n * 512:(n + 1) * 512], ps)
                return 2750

            vdone = set()
            ocount = [0]

            def vproj_unit(s8, n):
                """V chunk (s8, n) into v_sb (parity-placed)."""
                if (s8, n) in vdone:
                    return 0
                vdone.add((s8, n))
                ps = ppsum.tile([128, 512], F32, tag="proj")
                for c in range(C6):
                    nc.tensor.matmul(
                        ps,
                        lhsT=ctx16[:, c, s8 * 128:(s8 + 1) * 128],
                        rhs=wv16[:, c, n * 512:(n + 1) * 512],
                        start=(c == 0), stop=(c == C6 - 1),
                    )
                psv = ps.rearrange("p (pr two d) -> p pr two d", two=2, d=D)
                pr = slice(n * 4, (n + 1) * 4)
                nc.vector.tensor_copy(v_sb[:, s8, pr, 0:D], psv[:, :, 0, :])
                nc.vector.tensor_copy(v_sb[:, s8, pr, 2 * D:3 * D], psv[:, :, 1, :])
                return 1400

            def qproj_unit(tn, e):
                """qts[tn%2][:, e, :] = (Wq^T tok)^T chunk."""
                qt = qts[tn % 2]
                tsl = slice(tn * 512, (tn + 1) * 512)
                ps = ppsum.tile([128, 512], F32, tag="proj")
                for k in range(K8):
                    nc.tensor.matmul(
                        ps,
                        lhsT=wq16[:, k, e * 128:(e + 1) * 128],
                        rhs=tok16[:, k, tsl],
                        start=(k == 0), stop=(k == K8 - 1),
                    )
                nc.vector.tensor_copy(qt[:, e, :], ps)
                return 1850

            def oproj_unit(tn, m, n):
                """out[tn*512+m*128 :, n*512 :] = ct^T.T @ Wo + bo."""
                ct = cts[tn % 2]
                ps = ppsum.tile([128, 512], F32, tag="proj")
                for e in range(E8):
                    nc.tensor.matmul(
                        ps,
                        lhsT=ct[:, e, m * 128:(m + 1) * 128],
                        rhs=wo16[:, e, n * 512:(n + 1) * 512],
                        start=(e == 0), stop=(e == E8 - 1),
                    )
                ot = ostage.tile([128, 512], F32, tag="ot")
                nc.vector.tensor_add(ot, ps, bo_bc[:, n * 512:(n + 1) * 512])
                ocount[0] += 1
                oeng = nc.sync if ocount[0] % 2 == 0 else nc.gpsimd
                oeng.dma_start(
                    out=out[tn * 512 + m * 128: tn * 512 + (m + 1) * 128,
                            n * 512:(n + 1) * 512],
                    in_=ot,
                )
                return 1850

            # ---- attention helpers ------------------------------------
            def scores_exp(qt, p, s8):
                """Two-head score matmuls (disjoint PE row halves) + exp."""
                sp = spsum.tile([128, 2, 512], F32, tag="sp")
                for par in range(2):
                    prow = slice(par * 64, par * 64 + 64)
                    nc.tensor.matmul(
                        sp[:, par, :],
                        lhsT=kT_sb[prow, p, s8 * 128:(s8 + 1) * 128],
                        rhs=qt[prow, p, :],
                        start=True, stop=True,
                    )
                at = attnp.tile([128, 2, 512], BF16, tag="at")
                nc.scalar.activation(at, sp, AF.Exp, scale=0.125)
                return at

            def pv_step(cps, at_tiles, p, s8):
                """One s8 accumulation step of the PV matmul, both parities."""
                for par in range(2):
                    nc.tensor.matmul(
                        cps[par],
                        lhsT=v_sb[:, s8, p, par * D:par * D + 128],
                        rhs=at_tiles[s8][:, par, :],
                        start=(s8 == 0), stop=(s8 == S8 - 1),
                        skip_group_check=True,
                    )

            def normalize(ct, cps, p):
                """ct[:, p, :] = ctx rows / sumexp (per parity)."""
                for par in range(2):
                    cp = cps[par]
                    crow = slice(par * 64, par * 64 + 64)
                    srow = slice(64 - par * 64, 128 - par * 64)
                    rbs = small.tile([128, 512], F32, tag="rbs")
                    # full-tile: the custom DVE op mis-addresses partition-
                    # offset APs; the ctx-half results are junk and get
                    # overwritten by the partition-shift DMA below
                    nc.vector.reciprocal_approx_fast(rbs, cp)
                    nc.gpsimd.dma_start(out=rbs[crow, :], in_=rbs[srow, :])
                    nc.vector.tensor_mul(ct[crow, p, :], cp[crow, :], rbs[crow, :])

            # ---- filler queue -----------------------------------------
            # (ready_gi, due_gi, cost_ns, fn). Emission order == Tile
            # program order, so due_gi is a correctness bound (producer
            # emitted before consumer), not just a perf hint.
            units = []
            for e in range(1, E8):
                units.append([0, e, 2750, lambda e=e: kproj_unit(e)])
            for e in range(1, E8):
                units.append([0, e, 1850, lambda e=e: qproj_unit(0, e)])
            for s8 in range(S8):
                units.append([1, NG - 1, 1400, lambda s8=s8: vproj_unit(s8, 0)])
            for s8 in range(S8):
                units.append([1, NG - 1, 1400, lambda s8=s8: vproj_unit(s8, 1)])
            for e in range(E8):
                units.append([0, 8 + e, 1850, lambda e=e: qproj_unit(1, e)])
            for tn in range(1, TN - 1):
                for e in range(E8):
                    units.append([tn * 8, (tn + 1) * 8 + e, 1850,
                                  lambda tn=tn, e=e: qproj_unit(tn + 1, e)])
            for tn in range(TN - 1):
                for i, (m, n) in enumerate((m, n) for m in range(4)
                                           for n in range(NH)):
                    if tn == TN - 2 and m == 3:
                        continue  # reserved as epilogue fillers (PE warm
                                  # through the final normalize latency)
                    units.append([(tn + 1) * 8 + 1, (tn + 1) * 8 + 1 + (i * 6) // 8,
                                  1850,
                                  lambda tn=tn, m=m, n=n: oproj_unit(tn, m, n)])
            units.sort(key=lambda u: (u[0], u[1]))

            active = []
            uidx = [0]
            credit = [0.0]

            def activate_ready(gi):
                while uidx[0] < len(units) and units[uidx[0]][0] <= gi:
                    active.append(units[uidx[0]])
                    uidx[0] += 1

            def force_due(gi):
                i = 0
                while i < len(active):
                    if active[i][1] <= gi:
                        u = active.pop(i)
                        credit[0] -= u[3]()
                    else:
                        i += 1

            def drain():
                while active and credit[0] >= active[0][2]:
                    u = active.pop(0)
                    credit[0] -= u[3]()

            # ---- prologue ---------------------------------------------
            kproj_unit(0)
            qproj_unit(0, 0)

            # ---- software-pipelined main loop -------------------------
            groups = [(tn, p) for tn in range(TN) for p in range(NP)]
            prev = None   # (p_prev, at_tiles, ct_prev)

            for gi, (tn, p) in enumerate(groups):
                activate_ready(gi)
                force_due(gi)
                qt = qts[tn % 2]
                ct = cts[tn % 2]
                cps = None
                if prev is not None:
                    cps = [cpsum.tile([128, 512], F32, tag="cp",
                                      name=f"cp_{tn}_{p}_{i}") for i in range(2)]
                ats = []
                for s8 in range(S8):
                    if s8 == 0:
                        ats.append(scores_exp(qt, p, 0))
                        credit[0] -= 343
                    if prev is not None:
                        credit[0] -= vproj_unit(s8, prev[0] // 4)
                        pv_step(cps, prev[1], prev[0], s8)
                        credit[0] -= 480
                    drain()
                    if s8 <= S8 - 2:
                        ats.append(scores_exp(qt, p, s8 + 1))
                        credit[0] -= 343
                    credit[0] += 1117
                    cap = 25000.0 if gi == 0 else 3000.0
                    credit[0] = min(credit[0], cap)
                if prev is not None:
                    normalize(prev[2], cps, prev[0])
                prev = (p, ats, ct)

            # ---- epilogue: last group's PV + final out-proj -----------
            # Per-parity split: the par-1 PV matmuls and the reserved
            # oproj(tn=2, m=3) units keep the PE warm (no HAM cool-down)
            # while the normalize reciprocal/shift latency drains.
            force_due(NG)
            cps = [cpsum.tile([128, 512], F32, tag="cp", name=f"cp_last_{i}")
                   for i in range(2)]
            p_l, ats_l, ct_l = prev
            for s8 in range(S8):
                nc.tensor.matmul(
                    cps[0], lhsT=v_sb[:, s8, p_l, 0:128],
                    rhs=ats_l[s8][:, 0, :],
                    start=(s8 == 0), stop=(s8 == S8 - 1),
                    skip_group_check=True)
            rbs0 = small.tile([128, 512], F32, tag="rbs")
            nc.vector.reciprocal_approx_fast(rbs0, cps[0])
            nc.gpsimd.dma_start(out=rbs0[0:64, :], in_=rbs0[64:128, :])
            for s8 in range(S8):
                nc.tensor.matmul(
                    cps[1], lhsT=v_sb[:, s8, p_l, D:D + 128],
                    rhs=ats_l[s8][:, 1, :],
                    start=(s8 == 0), stop=(s8 == S8 - 1),
                    skip_group_check=True)
            nc.vector.tensor_mul(ct_l[0:64, p_l, :], cps[0][0:64, :],
                                 rbs0[0:64, :])
            oproj_unit(TN - 2, 3, 0)
            rbs1 = small.tile([128, 512], F32, tag="rbs")
            nc.vector.reciprocal_approx_fast(rbs1, cps[1])
            nc.gpsimd.dma_start(out=rbs1[64:128, :], in_=rbs1[0:64, :])
            oproj_unit(TN - 2, 3, 1)
            nc.vector.tensor_mul(ct_l[64:128, p_l, :], cps[1][64:128, :],
                                 rbs1[64:128, :])
            for m in range(4):
                for n in range(NH):
                    oproj_unit(TN - 1, m, n)

    nc.compile()
    return nc


def _get_nc():
    if "nc" not in _BUILT:
        _BUILT["nc"] = _build_nc()
    return _BUILT["nc"]


def _bf16(x):
    return np.asarray(x, dtype=np.float32).astype(ml_dtypes.bfloat16)


def kernel(tokens, context, Wq, Wk, Wv, Wo, bo):
    tokens = np.asarray(tokens, dtype=np.float32).reshape(B * T, HID)
    context = np.asarray(context, dtype=np.float32)
    bo2 = _bf16(np.asarray(bo, dtype=np.float32).reshape(1, HID))
    wq_b, wk_b, wv_b, wo_b = _bf16(Wq), _bf16(Wk), _bf16(Wv), _bf16(Wo)

    in_maps = []
    for c in range(N_CORES):
        b = (c * TC) // T
        tok_slice = tokens[c * TC:(c + 1) * TC, :]
        in_maps.append({
            "tokT": np.ascontiguousarray(tok_slice.T).astype(ml_dtypes.bfloat16),
            "ctxT": np.ascontiguousarray(context[b].T).astype(ml_dtypes.bfloat16),
            "wq": wq_b, "wk": wk_b, "wv": wv_b, "wo": wo_b, "bo": bo2,
        })

    nc = _get_nc()
    res = run_bass_kernel_spmd(nc, in_maps, list(range(N_CORES)))
    out = np.concatenate([res.results[c]["out"] for c in range(N_CORES)], axis=0)
    return out.reshape(B, T, HID)


# revision 16
# speedup vs baseline: 1.0225x; 1.0225x over previous
"""Cross-attention Trainium2 kernel (8 NeuronCores, SPMD, no collectives).

Reference computation (f32):
    Q = tokens @ Wq; K = context @ Wk; V = context @ Wv    (per batch)
    attn = softmax(Q K^T / sqrt(64)); ctx = attn V; out = ctx @ Wo + bo

Sharding: the flattened (B*T = 16384) token rows are split into 8 slices of
2048; each slice lies inside a single batch, so each core computes its own
batch's K/V locally (context is small) and needs no cross-core traffic.

Layouts: all activations contraction-major; scores transposed ([S, T]); the
softmax row-sum comes from a ones-column appended to V.  All matmul inputs
bf16 (fp8 was measured at 2-3e-2 max-rel error per stage -- over budget).

v4 schedule: the PE is the bottleneck (~375us of issue slots: scores 88,
PV 111, projections 152, K/V 42).  The Scalar engine (exp, 286us) runs
underneath.  All projection matmuls are fillers packed into the score/PV
stream via a ns-credit-paced queue whose due-dates double as emission-order
(=Tile dependency) correctness bounds; scores are emitted one chunk ahead
so ACT keeps >=1 chunk of backlog across group boundaries; inputs stream in
tn-chunk order on three parallel DMA rings so the first score matmul can
issue ~6us after launch.
"""

import numpy as np
import ml_dtypes

import concourse.bass as bass
import concourse.mybir as mybir
import concourse.tile as tile
from concourse import bacc
from concourse.bass_utils import run_bass_kernel_spmd

# problem shapes (hardcoded per the contract)
B, T, S = 4, 4096, 1024
HID, EMB, CTX, H = 1024, 1024, 768, 16
D = EMB // H  # 64
N_CORES = 8
TC = (B * T) // N_CORES  # 2048 token rows per core

F32 = mybir.dt.float32
BF16 = mybir.dt.bfloat16
AF = mybir.ActivationFunctionType

_BUILT = {}


def _build_nc(tc_=TC, s_=S, ctx_=CTX, hid_=HID, h_=H, num_cores=N_CORES):
    nc = bacc.Bacc("TRN2", target_bir_lowering=False, debug=False,
                   num_devices=num_cores)
    emb_ = hid_

    tokT = nc.dram_tensor("tokT", [hid_, tc_], BF16, kind="ExternalInput")
    ctxT = nc.dram_tensor("ctxT", [ctx_, s_], BF16, kind="ExternalInput")
    wq = nc.dram_tensor("wq", [hid_, emb_], BF16, kind="ExternalInput")
    wk = nc.dram_tensor("wk", [ctx_, emb_], BF16, kind="ExternalInput")
    wv = nc.dram_tensor("wv", [ctx_, hid_], BF16, kind="ExternalInput")
    wo = nc.dram_tensor("wo", [emb_, hid_], BF16, kind="ExternalInput")
    bo = nc.dram_tensor("bo", [1, hid_], BF16, kind="ExternalInput")
    out = nc.dram_tensor("out", [tc_, hid_], F32, kind="ExternalOutput")

    K8 = hid_ // 128   # contraction chunks for Q proj
    C6 = ctx_ // 128   # contraction chunks for K/V proj
    E8 = emb_ // 128   # embedding chunks
    S8 = s_ // 128     # source-sequence chunks
    TN = tc_ // 512    # T chunks per core
    NH = hid_ // 512   # output free chunks
    HID, EMB, CTX, S, TC, H = hid_, emb_, ctx_, s_, tc_, h_  # noqa: shadow
    NP = H // 2        # head pairs per T chunk
    NG = TN * NP       # total groups

    with tile.TileContext(nc) as tc:
        with (
            tc.tile_pool(name="const", bufs=1) as const,
            tc.tile_pool(name="qpool", bufs=2) as qpool,
            tc.tile_pool(name="cpool", bufs=2) as cpool,
            tc.tile_pool(name="attn", bufs=10) as attnp,
            tc.tile_pool(name="small", bufs=2) as small,
            tc.tile_pool(name="ostage", bufs=2) as ostage,
            tc.tile_pool(name="spsum", bufs=2, space="PSUM") as spsum,
            tc.tile_pool(name="cpsum", bufs=2, space="PSUM") as cpsum,
            tc.tile_pool(name="ppsum", bufs=2, space="PSUM") as ppsum,
        ):
            # ---- resident inputs (single big tiles, 2 DMA rings) ------
            # The Scalar engine must stay free for exp ACTIVATEs, so no DMA
            # triggers ride it.  The two prologue dependency sets are split
            # across the rings so both land ~simultaneously (~21us):
            # sync: {ctxT, wk} (kproj deps) then tokens tn1-3;
            # gpsimd: {wq, tokens-tn0} (qproj deps) then wv, wo, bo.
            ctx16 = const.tile([128, C6, S], BF16)
            for c in range(C6):
                nc.sync.dma_start(out=ctx16[:, c, :],
                                  in_=ctxT[c * 128:(c + 1) * 128, :])
            wk16 = const.tile([128, C6, EMB], BF16)
            for c in range(C6):
                nc.sync.dma_start(out=wk16[:, c, :],
                                  in_=wk[c * 128:(c + 1) * 128, :])
            wq16 = const.tile([128, K8, EMB], BF16)
            for k in range(K8):
                nc.gpsimd.dma_start(out=wq16[:, k, :],
                                    in_=wq[k * 128:(k + 1) * 128, :])
            tok16 = const.tile([128, K8, TC], BF16)
            for k in range(K8):
                nc.gpsimd.dma_start(
                    out=tok16[:, k, 0:512],
                    in_=tokT[k * 128:(k + 1) * 128, 0:512])
            for tn in range(1, TN):
                for k in range(K8):
                    nc.sync.dma_start(
                        out=tok16[:, k, tn * 512:(tn + 1) * 512],
                        in_=tokT[k * 128:(k + 1) * 128,
                                 tn * 512:(tn + 1) * 512])
            wv16 = const.tile([128, C6, HID], BF16)
            for c in range(C6):
                nc.gpsimd.dma_start(out=wv16[:, c, :],
                                    in_=wv[c * 128:(c + 1) * 128, :])
            wo16 = const.tile([128, E8, HID], BF16)
            for e in range(E8):
                nc.gpsimd.dma_start(out=wo16[:, e, :],
                                    in_=wo[e * 128:(e + 1) * 128, :])
            bo_bc = const.tile([128, HID], BF16)
            nc.gpsimd.dma_start(out=bo_bc, in_=bo[0:1, :].to_broadcast((128, HID)))

            # ---- SBUF result tiles ------------------------------------
            kT_sb = const.tile([128, E8, S], BF16)
            # V layout: [128, s8, pair, 192] = [V_par0 | ones | V_par1];
            # par-p lhsT is the overlapping window cols par*64 .. par*64+128,
            # so both parities share one ones block
            v_sb = const.tile([128, S8, NP, 3 * D], BF16)
            for s in range(S8):
                nc.vector.memset(v_sb[:, s, :, D:2 * D], 1.0)

            qts = [qpool.tile([128, E8, 512], BF16, tag="qt", name=f"qt{i}")
                   for i in range(2)]
            cts = [cpool.tile([128, E8, 512], BF16, tag="ct", name=f"ct{i}")
                   for i in range(2)]

            # ---- work units (cost in ~ns of PE issue time) ------------
            def kproj_unit(e):
                """kT_sb[:, e, :] = (context @ Wk)^T e-chunk."""
                for n in range(S // 512):
                    ps = ppsum.tile([128, 512], F32, tag="proj")
                    for c in range(C6):
                        nc.tensor.matmul(
                            ps,
                            lhsT=wk16[:, c, e * 128:(e + 1) * 128],
                            rhs=ctx16[:, c, n * 512:(n + 1) * 512],
                            start=(c == 0), stop=(c == C6 - 1),
                        )
                    nc.vector.tensor_copy(kT_sb[:, e, n * 512:(n + 1) * 512], ps)
                return 2750

            vdone = set()
            otail = [False, 0]

            def vproj_unit(s8, n):
                """V chunk (s8, n) into v_sb (parity-placed)."""
                if (s8, n) in vdone:
                    return 0
                vdone.add((s8, n))
                ps = ppsum.tile([128, 512], F32, tag="proj")
                for c in range(C6):
                    nc.tensor.matmul(
                        ps,
                        lhsT=ctx16[:, c, s8 * 128:(s8 + 1) * 128],
                        rhs=wv16[:, c, n * 512:(n + 1) * 512],
                        start=(c == 0), stop=(c == C6 - 1),
                    )
                psv = ps.rearrange("p (pr two d) -> p pr two d", two=2, d=D)
                pr = slice(n * 4, (n + 1) * 4)
                nc.vector.tensor_copy(v_sb[:, s8, pr, 0:D], psv[:, :, 0, :])
                nc.vector.tensor_copy(v_sb[:, s8, pr, 2 * D:3 * D], psv[:, :, 1, :])
                return 1400

            def qproj_unit(tn, e):
                """qts[tn%2][:, e, :] = (Wq^T tok)^T chunk."""
                qt = qts[tn % 2]
                tsl = slice(tn * 512, (tn + 1) * 512)
                ps = ppsum.tile([128, 512], F32, tag="proj")
                for k in range(K8):
                    nc.tensor.matmul(
                        ps,
                        lhsT=wq16[:, k, e * 128:(e + 1) * 128],
                        rhs=tok16[:, k, tsl],
                        start=(k == 0), stop=(k == K8 - 1),
                    )
                nc.vector.tensor_copy(qt[:, e, :], ps)
                return 1850

            def oproj_unit(tn, m, n):
                """out[tn*512+m*128 :, n*512 :] = ct^T.T @ Wo + bo."""
                ct = cts[tn % 2]
                ps = ppsum.tile([128, 512], F32, tag="proj")
                for e in range(E8):
                    nc.tensor.matmul(
                        ps,
                        lhsT=ct[:, e, m * 128:(m + 1) * 128],
                        rhs=wo16[:, e, n * 512:(n + 1) * 512],
                        start=(e == 0), stop=(e == E8 - 1),
                    )
                ot = ostage.tile([128, 512], F32, tag="ot")
                nc.vector.tensor_add(ot, ps, bo_bc[:, n * 512:(n + 1) * 512])
                otail[1] += 1
                oeng = nc.gpsimd if otail[0] and otail[1] % 2 else nc.sync
                oeng.dma_start(
                    out=out[tn * 512 + m * 128: tn * 512 + (m + 1) * 128,
                            n * 512:(n + 1) * 512],
                    in_=ot,
                )
                return 1850

            # ---- attention helpers ------------------------------------
            def scores_exp(qt, p, s8):
                """Two-head score matmuls (disjoint PE row halves) + exp."""
                sp = spsum.tile([128, 2, 512], F32, tag="sp")
                for par in range(2):
                    prow = slice(par * 64, par * 64 + 64)
                    nc.tensor.matmul(
                        sp[:, par, :],
                        lhsT=kT_sb[prow, p, s8 * 128:(s8 + 1) * 128],
                        rhs=qt[prow, p, :],
                        start=True, stop=True,
                    )
                at = attnp.tile([128, 2, 512], BF16, tag="at")
                nc.scalar.activation(at, sp, AF.Exp, scale=0.125)
                return at

            def pv_step(cps, at_tiles, p, s8):
                """One s8 accumulation step of the PV matmul, both parities."""
                for par in range(2):
                    nc.tensor.matmul(
                        cps[par],
                        lhsT=v_sb[:, s8, p, par * D:par * D + 128],
                        rhs=at_tiles[s8][:, par, :],
                        start=(s8 == 0), stop=(s8 == S8 - 1),
                        skip_group_check=True,
                    )

            def normalize(ct, cps, p):
                """ct[:, p, :] = ctx rows / sumexp (per parity)."""
                for par in range(2):
                    cp = cps[par]
                    crow = slice(par * 64, par * 64 + 64)
                    srow = slice(64 - par * 64, 128 - par * 64)
                    rbs = small.tile([128, 512], F32, tag="rbs")
                    # full-tile: the custom DVE op mis-addresses partition-
                    # offset APs; the ctx-half results are junk and get
                    # overwritten by the partition-shift DMA below
                    nc.vector.reciprocal_approx_fast(rbs, cp)
                    nc.sync.dma_start(out=rbs[crow, :], in_=rbs[srow, :])
                    nc.vector.tensor_mul(ct[crow, p, :], cp[crow, :], rbs[crow, :])

            # ---- filler queue -----------------------------------------
            # (ready_gi, due_gi, cost_ns, fn). Emission order == Tile
            # program order, so due_gi is a correctness bound (producer
            # emitted before consumer), not just a perf hint.
            units = []
            for e in range(1, E8):
                units.append([0, e, 2750, lambda e=e: kproj_unit(e)])
            for e in range(1, E8):
                units.append([0, e, 1850, lambda e=e: qproj_unit(0, e)])
            for s8 in range(S8):
                units.append([1, NG - 1, 1400, lambda s8=s8: vproj_unit(s8, 0)])
            for s8 in range(S8):
                units.append([1, NG - 1, 1400, lambda s8=s8: vproj_unit(s8, 1)])
            for e in range(E8):
                units.append([0, 8 + e, 1850, lambda e=e: qproj_unit(1, e)])
            for tn in range(1, TN - 1):
                for e in range(E8):
                    units.append([tn * 8, (tn + 1) * 8 + e, 1850,
                                  lambda tn=tn, e=e: qproj_unit(tn + 1, e)])
            for tn in range(TN - 1):
                for i, (m, n) in enumerate((m, n) for m in range(4)
                                           for n in range(NH)):
                    if tn == TN - 2 and m == 3:
                        continue  # reserved as epilogue fillers (PE warm
                                  # through the final normalize latency)
                    units.append([(tn + 1) * 8 + 1, (tn + 1) * 8 + 1 + (i * 6) // 8,
                                  1850,
                                  lambda tn=tn, m=m, n=n: oproj_unit(tn, m, n)])
            units.sort(key=lambda u: (u[0], u[1]))

            active = []
            uidx = [0]
            credit = [0.0]

            def activate_ready(gi):
                while uidx[0] < len(units) and units[uidx[0]][0] <= gi:
                    active.append(units[uidx[0]])
                    uidx[0] += 1

            def force_due(gi):
                i = 0
                while i < len(active):
                    if active[i][1] <= gi:
                        u = active.pop(i)
                        credit[0] -= u[3]()
                    else:
                        i += 1

            def drain():
                while active and credit[0] >= active[0][2]:
                    u = active.pop(0)
                    credit[0] -= u[3]()

            # ---- prologue ---------------------------------------------
            qproj_unit(0, 0)
            kproj_unit(0)

            # ---- software-pipelined main loop -------------------------
            groups = [(tn, p) for tn in range(TN) for p in range(NP)]
            prev = None   # (p_prev, at_tiles, ct_prev)

            for gi, (tn, p) in enumerate(groups):
                activate_ready(gi)
                force_due(gi)
                qt = qts[tn % 2]
                ct = cts[tn % 2]
                cps = None
                if prev is not None:
                    cps = [cpsum.tile([128, 512], F32, tag="cp",
                                      name=f"cp_{tn}_{p}_{i}") for i in range(2)]
                ats = [scores_exp(qt, p, 0), scores_exp(qt, p, 1)]
                credit[0] -= 686
                for js in range(S8 // 2):
                    if prev is not None:
                        for s8 in (2 * js, 2 * js + 1):
                            credit[0] -= vproj_unit(s8, prev[0] // 4)
                            pv_step(cps, prev[1], prev[0], s8)
                            credit[0] -= 480
                    drain()
                    if js < S8 // 2 - 1:
                        ats.append(scores_exp(qt, p, 2 * js + 2))
                        ats.append(scores_exp(qt, p, 2 * js + 3))
                        credit[0] -= 686
                    credit[0] += 2234
                    cap = 25000.0 if gi == 0 else 3400.0
                    credit[0] = min(credit[0], cap)
                if prev is not None:
                    normalize(prev[2], cps, prev[0])
                prev = (p, ats, ct)

            # ---- epilogue: last group's PV + final out-proj -----------
            # Per-parity split: the par-1 PV matmuls and the reserved
            # oproj(tn=2, m=3) units keep the PE warm (no HAM cool-down)
            # while the normalize reciprocal/shift latency drains.
            force_due(NG)
            cps = [cpsum.tile([128, 512], F32, tag="cp", name=f"cp_last_{i}")
                   for i in range(2)]
            p_l, ats_l, ct_l = prev
            for s8 in range(S8):
                nc.tensor.matmul(
                    cps[0], lhsT=v_sb[:, s8, p_l, 0:128],
                    rhs=ats_l[s8][:, 0, :],
                    start=(s8 == 0), stop=(s8 == S8 - 1),
                    skip_group_check=True)
            rbs0 = small.tile([128, 512], F32, tag="rbs")
            nc.vector.reciprocal_approx_fast(rbs0, cps[0])
            nc.sync.dma_start(out=rbs0[0:64, :], in_=rbs0[64:128, :])
            for s8 in range(S8):
                nc.tensor.matmul(
                    cps[1], lhsT=v_sb[:, s8, p_l, D:D + 128],
                    rhs=ats_l[s8][:, 1, :],
                    start=(s8 == 0), stop=(s8 == S8 - 1),
                    skip_group_check=True)
            nc.vector.tensor_mul(ct_l[0:64, p_l, :], cps[0][0:64, :],
                                 rbs0[0:64, :])
            oproj_unit(TN - 2, 3, 0)
            rbs1 = small.tile([128, 512], F32, tag="rbs")
            nc.vector.reciprocal_approx_fast(rbs1, cps[1])
            nc.sync.dma_start(out=rbs1[64:128, :], in_=rbs1[0:64, :])
            oproj_unit(TN - 2, 3, 1)
            nc.vector.tensor_mul(ct_l[64:128, p_l, :], cps[1][64:128, :],
                                 rbs1[64:128, :])
            otail[0] = True
            for m in range(4):
                for n in range(NH):
                    oproj_unit(TN - 1, m, n)

    nc.compile()
    return nc


def _get_nc():
    if "nc" not in _BUILT:
        _BUILT["nc"] = _build_nc()
    return _BUILT["nc"]


def _bf16(x):
    return np.asarray(x, dtype=np.float32).astype(ml_dtypes.bfloat16)


def kernel(tokens, context, Wq, Wk, Wv, Wo, bo):
    tokens = np.asarray(tokens, dtype=np.float32).reshape(B * T, HID)
    context = np.asarray(context, dtype=np.float32)
    bo2 = _bf16(np.asarray(bo, dtype=np.float32).reshape(1, HID))
    wq_b, wk_b, wv_b, wo_b = _bf16(Wq), _bf16(Wk), _bf16(Wv), _bf16(Wo)

    in_maps = []
    for c in range(N_CORES):
        b = (c * TC) // T
        tok_slice = tokens[c * TC:(c + 1) * TC, :]
        in_maps.append({
            "tokT": np.ascontiguousarray(tok_slice.T).astype(ml_dtypes.bfloat16),
            "ctxT": np.ascontiguousarray(context[b].T).astype(ml_dtypes.bfloat16),
            "wq": wq_b, "wk": wk_b, "wv": wv_b, "wo": wo_b, "bo": bo2,
        })

    nc = _get_nc()
    res = run_bass_kernel_spmd(nc, in_maps, list(range(N_CORES)))
    out = np.concatenate([res.results[c]["out"] for c in range(N_CORES)], axis=0)
    return out.reshape(B, T, HID)


# revision 17
# speedup vs baseline: 1.0226x; 1.0001x over previous
"""Cross-attention Trainium2 kernel (8 NeuronCores, SPMD, no collectives).

Reference computation (f32):
    Q = tokens @ Wq; K = context @ Wk; V = context @ Wv    (per batch)
    attn = softmax(Q K^T / sqrt(64)); ctx = attn V; out = ctx @ Wo + bo

Sharding: the flattened (B*T = 16384) token rows are split into 8 slices of
2048; each slice lies inside a single batch, so each core computes its own
batch's K/V locally (context is small) and needs no cross-core traffic.

Layouts: all activations contraction-major; scores transposed ([S, T]); the
softmax row-sum comes from a ones-column appended to V.  All matmul inputs
bf16 (fp8 was measured at 2-3e-2 max-rel error per stage -- over budget).

v4 schedule: the PE is the bottleneck (~375us of issue slots: scores 88,
PV 111, projections 152, K/V 42).  The Scalar engine (exp, 286us) runs
underneath.  All projection matmuls are fillers packed into the score/PV
stream via a ns-credit-paced queue whose due-dates double as emission-order
(=Tile dependency) correctness bounds; scores are emitted one chunk ahead
so ACT keeps >=1 chunk of backlog across group boundaries; inputs stream in
tn-chunk order on three parallel DMA rings so the first score matmul can
issue ~6us after launch.
"""

import numpy as np
import ml_dtypes

import concourse.bass as bass
import concourse.mybir as mybir
import concourse.tile as tile
from concourse import bacc
from concourse.bass_utils import run_bass_kernel_spmd

# problem shapes (hardcoded per the contract)
B, T, S = 4, 4096, 1024
HID, EMB, CTX, H = 1024, 1024, 768, 16
D = EMB // H  # 64
N_CORES = 8
TC = (B * T) // N_CORES  # 2048 token rows per core

F32 = mybir.dt.float32
BF16 = mybir.dt.bfloat16
AF = mybir.ActivationFunctionType

_BUILT = {}


def _build_nc(tc_=TC, s_=S, ctx_=CTX, hid_=HID, h_=H, num_cores=N_CORES):
    nc = bacc.Bacc("TRN2", target_bir_lowering=False, debug=False,
                   num_devices=num_cores)
    emb_ = hid_

    tokT = nc.dram_tensor("tokT", [hid_, tc_], BF16, kind="ExternalInput")
    ctxT = nc.dram_tensor("ctxT", [ctx_, s_], BF16, kind="ExternalInput")
    wq = nc.dram_tensor("wq", [hid_, emb_], BF16, kind="ExternalInput")
    wk = nc.dram_tensor("wk", [ctx_, emb_], BF16, kind="ExternalInput")
    wv = nc.dram_tensor("wv", [ctx_, hid_], BF16, kind="ExternalInput")
    wo = nc.dram_tensor("wo", [emb_, hid_], BF16, kind="ExternalInput")
    bo = nc.dram_tensor("bo", [1, hid_], BF16, kind="ExternalInput")
    out = nc.dram_tensor("out", [tc_, hid_], F32, kind="ExternalOutput")

    K8 = hid_ // 128   # contraction chunks for Q proj
    C6 = ctx_ // 128   # contraction chunks for K/V proj
    E8 = emb_ // 128   # embedding chunks
    S8 = s_ // 128     # source-sequence chunks
    TN = tc_ // 512    # T chunks per core
    NH = hid_ // 512   # output free chunks
    HID, EMB, CTX, S, TC, H = hid_, emb_, ctx_, s_, tc_, h_  # noqa: shadow
    NP = H // 2        # head pairs per T chunk
    NG = TN * NP       # total groups

    with tile.TileContext(nc) as tc:
        with (
            tc.tile_pool(name="const", bufs=1) as const,
            tc.tile_pool(name="qpool", bufs=2) as qpool,
            tc.tile_pool(name="cpool", bufs=2) as cpool,
            tc.tile_pool(name="attn", bufs=10) as attnp,
            tc.tile_pool(name="small", bufs=2) as small,
            tc.tile_pool(name="ostage", bufs=2) as ostage,
            tc.tile_pool(name="spsum", bufs=2, space="PSUM") as spsum,
            tc.tile_pool(name="cpsum", bufs=2, space="PSUM") as cpsum,
            tc.tile_pool(name="ppsum", bufs=2, space="PSUM") as ppsum,
        ):
            # ---- resident inputs (single big tiles, 3 DMA rings) ------
            # Prologue deps spread over three rings: sync {ctx, wv, tok
            # tn1-3}, gpsimd {wk, tok-tn0, wo}, scalar {wq ONLY -- its 8
            # trigger instructions retire ~5us in, long before the first
            # exp ACTIVATE (~20us), so the Scalar engine never blocks}.
            # kproj deps (ctx, wk) land ~11us, qproj deps (wq, tok0) ~18us,
            # wv ~21us (first PV needs it ~29us).
            ctx16 = const.tile([128, C6, S], BF16)
            for c in range(C6):
                nc.sync.dma_start(out=ctx16[:, c, :],
                                  in_=ctxT[c * 128:(c + 1) * 128, :])
            wk16 = const.tile([128, C6, EMB], BF16)
            for c in range(C6):
                nc.gpsimd.dma_start(out=wk16[:, c, :],
                                    in_=wk[c * 128:(c + 1) * 128, :])
            wq16 = const.tile([128, K8, EMB], BF16)
            for k in range(K8):
                nc.scalar.dma_start(out=wq16[:, k, :],
                                    in_=wq[k * 128:(k + 1) * 128, :])
            tok16 = const.tile([128, K8, TC], BF16)
            for k in range(K8):
                nc.gpsimd.dma_start(
                    out=tok16[:, k, 0:512],
                    in_=tokT[k * 128:(k + 1) * 128, 0:512])
            wv16 = const.tile([128, C6, HID], BF16)
            for c in range(C6):
                nc.sync.dma_start(out=wv16[:, c, :],
                                  in_=wv[c * 128:(c + 1) * 128, :])
            for tn in range(1, TN):
                for k in range(K8):
                    nc.sync.dma_start(
                        out=tok16[:, k, tn * 512:(tn + 1) * 512],
                        in_=tokT[k * 128:(k + 1) * 128,
                                 tn * 512:(tn + 1) * 512])
            wo16 = const.tile([128, E8, HID], BF16)
            for e in range(E8):
                nc.gpsimd.dma_start(out=wo16[:, e, :],
                                    in_=wo[e * 128:(e + 1) * 128, :])
            bo_bc = const.tile([128, HID], BF16)
            nc.gpsimd.dma_start(out=bo_bc, in_=bo[0:1, :].to_broadcast((128, HID)))

            # ---- SBUF result tiles ------------------------------------
            kT_sb = const.tile([128, E8, S], BF16)
            # V layout: [128, s8, pair, 192] = [V_par0 | ones | V_par1];
            # par-p lhsT is the overlapping window cols par*64 .. par*64+128,
            # so both parities share one ones block
            v_sb = const.tile([128, S8, NP, 3 * D], BF16)
            for s in range(S8):
                nc.vector.memset(v_sb[:, s, :, D:2 * D], 1.0)

            qts = [qpool.tile([128, E8, 512], BF16, tag="qt", name=f"qt{i}")
                   for i in range(2)]
            cts = [cpool.tile([128, E8, 512], BF16, tag="ct", name=f"ct{i}")
                   for i in range(2)]

            # ---- work units (cost in ~ns of PE issue time) ------------
            def kproj_unit(e):
                """kT_sb[:, e, :] = (context @ Wk)^T e-chunk."""
                for n in range(S // 512):
                    ps = ppsum.tile([128, 512], F32, tag="proj")
                    for c in range(C6):
                        nc.tensor.matmul(
                            ps,
                            lhsT=wk16[:, c, e * 128:(e + 1) * 128],
                            rhs=ctx16[:, c, n * 512:(n + 1) * 512],
                            start=(c == 0), stop=(c == C6 - 1),
                        )
                    nc.vector.tensor_copy(kT_sb[:, e, n * 512:(n + 1) * 512], ps)
                return 2750

            vdone = set()
            otail = [False, 0]

            def vproj_unit(s8, n):
                """V chunk (s8, n) into v_sb (parity-placed)."""
                if (s8, n) in vdone:
                    return 0
                vdone.add((s8, n))
                ps = ppsum.tile([128, 512], F32, tag="proj")
                for c in range(C6):
                    nc.tensor.matmul(
                        ps,
                        lhsT=ctx16[:, c, s8 * 128:(s8 + 1) * 128],
                        rhs=wv16[:, c, n * 512:(n + 1) * 512],
                        start=(c == 0), stop=(c == C6 - 1),
                    )
                psv = ps.rearrange("p (pr two d) -> p pr two d", two=2, d=D)
                pr = slice(n * 4, (n + 1) * 4)
                nc.vector.tensor_copy(v_sb[:, s8, pr, 0:D], psv[:, :, 0, :])
                nc.vector.tensor_copy(v_sb[:, s8, pr, 2 * D:3 * D], psv[:, :, 1, :])
                return 1400

            def qproj_unit(tn, e):
                """qts[tn%2][:, e, :] = (Wq^T tok)^T chunk."""
                qt = qts[tn % 2]
                tsl = slice(tn * 512, (tn + 1) * 512)
                ps = ppsum.tile([128, 512], F32, tag="proj")
                for k in range(K8):
                    nc.tensor.matmul(
                        ps,
                        lhsT=wq16[:, k, e * 128:(e + 1) * 128],
                        rhs=tok16[:, k, tsl],
                        start=(k == 0), stop=(k == K8 - 1),
                    )
                nc.vector.tensor_copy(qt[:, e, :], ps)
                return 1850

            def oproj_unit(tn, m, n):
                """out[tn*512+m*128 :, n*512 :] = ct^T.T @ Wo + bo."""
                ct = cts[tn % 2]
                ps = ppsum.tile([128, 512], F32, tag="proj")
                for e in range(E8):
                    nc.tensor.matmul(
                        ps,
                        lhsT=ct[:, e, m * 128:(m + 1) * 128],
                        rhs=wo16[:, e, n * 512:(n + 1) * 512],
                        start=(e == 0), stop=(e == E8 - 1),
                    )
                ot = ostage.tile([128, 512], F32, tag="ot")
                nc.vector.tensor_add(ot, ps, bo_bc[:, n * 512:(n + 1) * 512])
                otail[1] += 1
                oeng = nc.gpsimd if otail[0] and otail[1] % 2 else nc.sync
                oeng.dma_start(
                    out=out[tn * 512 + m * 128: tn * 512 + (m + 1) * 128,
                            n * 512:(n + 1) * 512],
                    in_=ot,
                )
                return 1850

            # ---- attention helpers ------------------------------------
            def scores_exp(qt, p, s8):
                """Two-head score matmuls (disjoint PE row halves) + exp."""
                sp = spsum.tile([128, 2, 512], F32, tag="sp")
                for par in range(2):
                    prow = slice(par * 64, par * 64 + 64)
                    nc.tensor.matmul(
                        sp[:, par, :],
                        lhsT=kT_sb[prow, p, s8 * 128:(s8 + 1) * 128],
                        rhs=qt[prow, p, :],
                        start=True, stop=True,
                    )
                at = attnp.tile([128, 2, 512], BF16, tag="at")
                nc.scalar.activation(at, sp, AF.Exp, scale=0.125)
                return at

            def pv_step(cps, at_tiles, p, s8):
                """One s8 accumulation step of the PV matmul, both parities."""
                for par in range(2):
                    nc.tensor.matmul(
                        cps[par],
                        lhsT=v_sb[:, s8, p, par * D:par * D + 128],
                        rhs=at_tiles[s8][:, par, :],
                        start=(s8 == 0), stop=(s8 == S8 - 1),
                        skip_group_check=True,
                    )

            def normalize(ct, cps, p):
                """ct[:, p, :] = ctx rows / sumexp (per parity)."""
                for par in range(2):
                    cp = cps[par]
                    crow = slice(par * 64, par * 64 + 64)
                    srow = slice(64 - par * 64, 128 - par * 64)
                    rbs = small.tile([128, 512], F32, tag="rbs")
                    # full-tile: the custom DVE op mis-addresses partition-
                    # offset APs; the ctx-half results are junk and get
                    # overwritten by the partition-shift DMA below
                    nc.vector.reciprocal_approx_fast(rbs, cp)
                    nc.sync.dma_start(out=rbs[crow, :], in_=rbs[srow, :])
                    nc.vector.tensor_mul(ct[crow, p, :], cp[crow, :], rbs[crow, :])

            # ---- filler queue -----------------------------------------
            # (ready_gi, due_gi, cost_ns, fn). Emission order == Tile
            # program order, so due_gi is a correctness bound (producer
            # emitted before consumer), not just a perf hint.
            units = []
            for e in range(1, E8):
                units.append([0, e, 2750, lambda e=e: kproj_unit(e)])
            for e in range(1, E8):
                units.append([0, e, 1850, lambda e=e: qproj_unit(0, e)])
            for s8 in range(S8):
                units.append([1, NG - 1, 1400, lambda s8=s8: vproj_unit(s8, 0)])
            for s8 in range(S8):
                units.append([1, NG - 1, 1400, lambda s8=s8: vproj_unit(s8, 1)])
            for e in range(E8):
                units.append([0, 8 + e, 1850, lambda e=e: qproj_unit(1, e)])
            for tn in range(1, TN - 1):
                for e in range(E8):
                    units.append([tn * 8, (tn + 1) * 8 + e, 1850,
                                  lambda tn=tn, e=e: qproj_unit(tn + 1, e)])
            for tn in range(TN - 1):
                for i, (m, n) in enumerate((m, n) for m in range(4)
                                           for n in range(NH)):
                    if tn == TN - 2 and m == 3:
                        continue  # reserved as epilogue fillers (PE warm
                                  # through the final normalize latency)
                    units.append([(tn + 1) * 8 + 1, (tn + 1) * 8 + 1 + (i * 6) // 8,
                                  1850,
                                  lambda tn=tn, m=m, n=n: oproj_unit(tn, m, n)])
            units.sort(key=lambda u: (u[0], u[1]))

            active = []
            uidx = [0]
            credit = [0.0]

            def activate_ready(gi):
                while uidx[0] < len(units) and units[uidx[0]][0] <= gi:
                    active.append(units[uidx[0]])
                    uidx[0] += 1

            def force_due(gi):
                i = 0
                while i < len(active):
                    if active[i][1] <= gi:
                        u = active.pop(i)
                        credit[0] -= u[3]()
                    else:
                        i += 1

            def drain():
                while active and credit[0] >= active[0][2]:
                    u = active.pop(0)
                    credit[0] -= u[3]()

            # ---- prologue ---------------------------------------------
            qproj_unit(0, 0)
            kproj_unit(0)

            # ---- software-pipelined main loop -------------------------
            groups = [(tn, p) for tn in range(TN) for p in range(NP)]
            prev = None   # (p_prev, at_tiles, ct_prev)

            for gi, (tn, p) in enumerate(groups):
                activate_ready(gi)
                force_due(gi)
                qt = qts[tn % 2]
                ct = cts[tn % 2]
                cps = None
                if prev is not None:
                    cps = [cpsum.tile([128, 512], F32, tag="cp",
                                      name=f"cp_{tn}_{p}_{i}") for i in range(2)]
                ats = [scores_exp(qt, p, 0), scores_exp(qt, p, 1)]
                credit[0] -= 686
                for js in range(S8 // 2):
                    if prev is not None:
                        for s8 in (2 * js, 2 * js + 1):
                            credit[0] -= vproj_unit(s8, prev[0] // 4)
                            pv_step(cps, prev[1], prev[0], s8)
                            credit[0] -= 480
                    drain()
                    if js < S8 // 2 - 1:
                        ats.append(scores_exp(qt, p, 2 * js + 2))
                        ats.append(scores_exp(qt, p, 2 * js + 3))
                        credit[0] -= 686
                    credit[0] += 2234
                    cap = 25000.0 if gi == 0 else 3400.0
                    credit[0] = min(credit[0], cap)
                if prev is not None:
                    normalize(prev[2], cps, prev[0])
                prev = (p, ats, ct)

            # ---- epilogue: last group's PV + final out-proj -----------
            # Per-parity split: the par-1 PV matmuls and the reserved
            # oproj(tn=2, m=3) units keep the PE warm (no HAM cool-down)
            # while the normalize reciprocal/shift latency drains.
            force_due(NG)
            cps = [cpsum.tile([128, 512], F32, tag="cp", name=f"cp_last_{i}")
                   for i in range(2)]
            p_l, ats_l, ct_l = prev
            for s8 in range(S8):
                nc.tensor.matmul(
                    cps[0], lhsT=v_sb[:, s8, p_l, 0:128],
                    rhs=ats_l[s8][:, 0, :],
                    start=(s8 == 0), stop=(s8 == S8 - 1),
                    skip_group_check=True)
            rbs0 = small.tile([128, 512], F32, tag="rbs")
            nc.vector.reciprocal_approx_fast(rbs0, cps[0])
            nc.sync.dma_start(out=rbs0[0:64, :], in_=rbs0[64:128, :])
            for s8 in range(S8):
                nc.tensor.matmul(
                    cps[1], lhsT=v_sb[:, s8, p_l, D:D + 128],
                    rhs=ats_l[s8][:, 1, :],
                    start=(s8 == 0), stop=(s8 == S8 - 1),
                    skip_group_check=True)
            nc.vector.tensor_mul(ct_l[0:64, p_l, :], cps[0][0:64, :],
                                 rbs0[0:64, :])
            oproj_unit(TN - 2, 3, 0)
            rbs1 = small.tile([128, 512], F32, tag="rbs")
            nc.vector.reciprocal_approx_fast(rbs1, cps[1])
            nc.sync.dma_start(out=rbs1[64:128, :], in_=rbs1[0:64, :])
            oproj_unit(TN - 2, 3, 1)
            nc.vector.tensor_mul(ct_l[64:128, p_l, :], cps[1][64:128, :],
                                 rbs1[64:128, :])
            otail[0] = True
            for m in range(4):
                for n in range(NH):
                    oproj_unit(TN - 1, m, n)

    nc.compile()
    return nc


def _get_nc():
    if "nc" not in _BUILT:
        _BUILT["nc"] = _build_nc()
    return _BUILT["nc"]


def _bf16(x):
    return np.asarray(x, dtype=np.float32).astype(ml_dtypes.bfloat16)


def kernel(tokens, context, Wq, Wk, Wv, Wo, bo):
    tokens = np.asarray(tokens, dtype=np.float32).reshape(B * T, HID)
    context = np.asarray(context, dtype=np.float32)
    bo2 = _bf16(np.asarray(bo, dtype=np.float32).reshape(1, HID))
    wq_b, wk_b, wv_b, wo_b = _bf16(Wq), _bf16(Wk), _bf16(Wv), _bf16(Wo)

    in_maps = []
    for c in range(N_CORES):
        b = (c * TC) // T
        tok_slice = tokens[c * TC:(c + 1) * TC, :]
        in_maps.append({
            "tokT": np.ascontiguousarray(tok_slice.T).astype(ml_dtypes.bfloat16),
            "ctxT": np.ascontiguousarray(context[b].T).astype(ml_dtypes.bfloat16),
            "wq": wq_b, "wk": wk_b, "wv": wv_b, "wo": wo_b, "bo": bo2,
        })

    nc = _get_nc()
    res = run_bass_kernel_spmd(nc, in_maps, list(range(N_CORES)))
    out = np.concatenate([res.results[c]["out"] for c in range(N_CORES)], axis=0)
    return out.reshape(B, T, HID)
